# revision 10
# baseline (speedup 1.0000x reference)
"""Multi-head attention Trainium2 kernel v3 (8 NeuronCores, SPMD).

Problem: B=4, S=2048, D=1024, H=16, HD=64 dense MHA with 0/1 mask applied
to scores BEFORE softmax (masked score -> 0, so exp -> 1).

Sharding: core c handles batch b = c//2 and HEAD-half hh = c%2 (8 heads
= 4 pairs), ALL 2048 queries and keys -> no K/V projection duplication.
Each core emits a PARTIAL output (its heads' z @ Wo rows); the host adds
the two partials per batch (bo included on hh=0 only).

Mask/softmax factoring:  E = m*exp(s) + (1-m) = (exp(s)-1)*m + 1.
Per kc chunk the Scalar engine computes exp(s) (unmasked), ONE DVE
scalar_tensor_tensor forms (exp(s)-1)*m in bf16 at the 2x DVE rate (the
mask block is read twice through a 0-stride broadcast AP to cover both
heads), and the attn@V matmuls consume that. The "+1" term is exact and
mask-independent: sum_k 1*V'[k,e] = [Vcolsum_h | S], computed on the
HOST and injected as the PSUM *initialization* of each zT accumulator
via a K=1 matmul (lhsT = vch column, rhs = ones). The ones column of V'
then yields the full denominator in zT row 64 with no extra work.

P-state discipline: the PE drops from 2.4GHz to 1.2GHz after ~1us of
idle and needs ~3us of continuous work to ramp back. The attention loop
keeps every PE gap under ~0.5us: zT PSUM accumulators are double
buffered (8/8 banks used), and each block's epilogue (den copy ->
broadcast -> fast reciprocal -> normalize) is EMITTED two iterations
into the NEXT block so it fills DVE/PE slack instead of stalling the
queues at block boundaries.
"""

import sys

sys.path.insert(0, "/opt/trn_rl_repo")

import numpy as np
import ml_dtypes

import concourse.bass as bass
import concourse.mybir as mybir
import concourse.tile as tile
from concourse import bacc
from concourse.bass_utils import run_bass_kernel_spmd

BF16 = ml_dtypes.bfloat16

B, S, D, H, HD = 4, 2048, 1024, 16, 64
HL = 8             # local heads per core
NPAIR = 4          # local head pairs
SK = 2048          # keys
QW = 2048          # queries per core (all)
KC = 16            # key chunks of 128
DC = 8             # contraction chunks of 128 over D
QB = 512           # query block per head in the paired scores tile
VW = 65            # V width incl. ones column
N_CORES = 8

_CACHED_NC = None


def _build_nc():
    dt = mybir.dt
    f32, b16 = dt.float32, dt.bfloat16
    Copy = mybir.ActivationFunctionType.Copy
    Ident = mybir.ActivationFunctionType.Identity
    Exp = mybir.ActivationFunctionType.Exp
    Alu = mybir.AluOpType

    nc = bacc.Bacc("TRN2", target_bir_lowering=False, debug=False)

    d_xqt = nc.dram_tensor("xqt", [D, QW], b16, kind="ExternalInput").ap()
    d_xkt = nc.dram_tensor("xkt", [D, SK], b16, kind="ExternalInput").ap()
    d_xvt = nc.dram_tensor("xvt", [D, SK], b16, kind="ExternalInput").ap()
    d_m = nc.dram_tensor("m", [SK, QW], b16, kind="ExternalInput").ap()
    d_wq = nc.dram_tensor("wq", [D, 512], b16, kind="ExternalInput").ap()
    d_wk = nc.dram_tensor("wk", [D, 512], b16, kind="ExternalInput").ap()
    d_wv = nc.dram_tensor("wv", [D, 512], b16, kind="ExternalInput").ap()
    d_wo = nc.dram_tensor("wo", [128, NPAIR * D], b16, kind="ExternalInput").ap()
    d_bq = nc.dram_tensor("bq", [128, NPAIR], f32, kind="ExternalInput").ap()
    d_bk = nc.dram_tensor("bk", [128, NPAIR], f32, kind="ExternalInput").ap()
    d_bv = nc.dram_tensor("bv", [1, 512], b16, kind="ExternalInput").ap()
    # per pair/head: [Vcolsum_h(64) | S] columns for the zT PSUM init
    d_vch = nc.dram_tensor("vch", [1, NPAIR * 2 * VW], b16, kind="ExternalInput").ap()
    d_bob = nc.dram_tensor("bob", [128, D], f32, kind="ExternalInput").ap()
    d_out = nc.dram_tensor("out", [QW, D], f32, kind="ExternalOutput").ap()

    with tile.TileContext(nc) as tc:
        _keep = []

        def single(shape, dtype, name):
            t, free = tc.tile(shape, dtype, name=name)
            done = [False]

            def free_once():
                if not done[0]:
                    done[0] = True
                    free()

            _keep.append(free_once)
            return t, free_once

        # ---------------- persistent SBUF tiles ----------------
        kt, _ = single([128, NPAIR * SK], b16, "kt")       # [pair-e, k]
        qt_, _ = single([128, NPAIR * QW], b16, "qt")      # [pair-e, q]
        vp, _ = single([128, KC * HL * VW], b16, "vp")     # [k-chunk, h*65]
        # mask ring: 2 slots of [k-chunk, one qb column block]
        m_sb, _ = single([128, 2 * KC * QB], b16, "m_sb")
        wo_sb, _ = single([128, NPAIR * D], b16, "wo_sb")
        zt, _ = single([128, NPAIR * QW], b16, "zt")       # [pair-he, q]
        ones1, _ = single([1, 128], b16, "ones1")
        onesr, _ = single([1, 512], b16, "onesr")
        bqp, _ = single([128, NPAIR], f32, "bqp")
        bkp, _ = single([128, NPAIR], f32, "bkp")
        bvr, _ = single([1, 512], b16, "bvr")
        vch_sb, _ = single([1, NPAIR * 2 * VW], b16, "vch_sb")
        bob_sb, _ = single([128, D], f32, "bob_sb")

        nc.vector.memset(ones1[:], 1.0)
        nc.vector.memset(onesr[:], 1.0)
        nc.vector.memset(vp[:, 64::65], 1.0)

        nc.sync.dma_start(bqp[:], d_bq[:])
        nc.sync.dma_start(bkp[:], d_bk[:])
        nc.sync.dma_start(bvr[:], d_bv[:])
        nc.sync.dma_start(vch_sb[:], d_vch[:])
        nc.sync.dma_start(bob_sb[:], d_bob[:])
        nc.sync.dma_start(wo_sb[:], d_wo[:])

        # Phase-chained staging buffers (LIFO frees).
        xk_sb, xk_free = single([128, DC * SK], b16, "xk_sb")
        wk_sb, wk_free = single([128, DC * 512], b16, "wk_sb")
        xv_sb, xv_free = single([128, DC * SK], b16, "xv_sb")
        wv_sb, wv_free = single([128, DC * 512], b16, "wv_sb")

        for dc in range(DC):
            nc.sync.dma_start(xv_sb[:, dc * SK:(dc + 1) * SK], d_xvt[dc * 128:(dc + 1) * 128, :])
            nc.sync.dma_start(wv_sb[:, dc * 512:(dc + 1) * 512], d_wv[dc * 128:(dc + 1) * 128, :])
        # mask for the first query block into ring slot 0, then K staging
        def dma_mask(qb):
            s = (qb % 2) * KC * QB
            for kc in range(KC):
                nc.sync.dma_start(
                    m_sb[:, s + kc * QB: s + (kc + 1) * QB],
                    d_m[kc * 128:(kc + 1) * 128, qb * QB:(qb + 1) * QB],
                )

        dma_mask(0)
        for dc in range(DC):
            nc.sync.dma_start(xk_sb[:, dc * SK:(dc + 1) * SK], d_xkt[dc * 128:(dc + 1) * 128, :])
            nc.sync.dma_start(wk_sb[:, dc * 512:(dc + 1) * 512], d_wk[dc * 128:(dc + 1) * 128, :])

        # ---------------- projections ----------------
        with tc.tile_pool(name="proj_ps", space="PSUM", bufs=4) as proj_pool:
            # V proj: V'[k, h*65+e] (+ones col)
            for sc in range(KC):
                ps = proj_pool.tile([128, 512], f32, tag="ps")
                nc.tensor.matmul(  # bias: ones[k] x bv[he]
                    ps[:], lhsT=ones1[:, 0:128], rhs=bvr[:],
                    start=True, stop=False,
                )
                for dc in range(DC):
                    nc.tensor.matmul(
                        ps[:],
                        lhsT=xv_sb[:, dc * SK + sc * 128: dc * SK + (sc + 1) * 128],
                        rhs=wv_sb[:, dc * 512:(dc + 1) * 512],
                        start=False, stop=(dc == DC - 1),
                    )
                o3 = vp[:, sc * HL * VW: (sc + 1) * HL * VW]
                o3 = o3.rearrange("p (h c) -> p h c", h=HL)[:, :, 0:64]
                i3 = ps[:].rearrange("p (h c) -> p h c", h=HL)
                nc.scalar.activation(o3, i3, Copy)
            wv_free()
            xv_free()

            # stage Q inputs now so their DMA overlaps K projection
            xq_sb, xq_free = single([128, DC * QW], b16, "xq_sb")
            wq_sb, wq_free = single([128, DC * 512], b16, "wq_sb")
            for dc in range(DC):
                nc.sync.dma_start(xq_sb[:, dc * QW:(dc + 1) * QW], d_xqt[dc * 128:(dc + 1) * 128, :])
                nc.sync.dma_start(wq_sb[:, dc * 512:(dc + 1) * 512], d_wq[dc * 128:(dc + 1) * 128, :])

            # K proj -> KT [pair-e 128, k]
            for p in range(NPAIR):
                for ns in range(SK // 512):
                    ps = proj_pool.tile([128, 512], f32, tag="ps")
                    for dc in range(DC):
                        nc.tensor.matmul(
                            ps[:],
                            lhsT=wk_sb[:, dc * 512 + p * 128: dc * 512 + (p + 1) * 128],
                            rhs=xk_sb[:, dc * SK + ns * 512: dc * SK + (ns + 1) * 512],
                            start=(dc == 0), stop=(dc == DC - 1),
                        )
                    nc.scalar.activation(
                        kt[:, p * SK + ns * 512: p * SK + (ns + 1) * 512],
                        ps[:], Ident, bias=bkp[:, p: p + 1],
                    )
            # Q proj -> QT [pair-e 128, q]
            for p in range(NPAIR):
                for ns in range(QW // 512):
                    ps = proj_pool.tile([128, 512], f32, tag="ps")
                    for dc in range(DC):
                        nc.tensor.matmul(
                            ps[:],
                            lhsT=wq_sb[:, dc * 512 + p * 128: dc * 512 + (p + 1) * 128],
                            rhs=xq_sb[:, dc * QW + ns * 512: dc * QW + (ns + 1) * 512],
                            start=(dc == 0), stop=(dc == DC - 1),
                        )
                    nc.scalar.activation(
                        qt_[:, p * QW + ns * 512: p * QW + (ns + 1) * 512],
                        ps[:], Ident, bias=bqp[:, p: p + 1],
                    )
            wq_free()
            xq_free()
            wk_free()
            xk_free()

        # ---------------- attention ----------------
        with (
            tc.tile_pool(name="sc_ps", space="PSUM", bufs=2) as sc_pool,
            tc.tile_pool(name="zt_ps", space="PSUM", bufs=2) as zt_pool,
            tc.tile_pool(name="att_sb", bufs=4) as att_pool,
            tc.tile_pool(name="ep_sb", bufs=2) as ep_pool,
        ):
            pending_ep = [None]

            def emit_epilogue_a0():
                # den rows -> SBUF early (kc==0 slot) so the later broadcast
                # +reciprocal hold the borrowed scores-ring PSUM slot only
                # briefly (PE bubble stays under the p-state drop threshold).
                st = pending_ep[0]
                if st is None or len(st) != 2:
                    return
                zps, q0p = st
                den_b = ep_pool.tile([1, 2 * QB], b16, name="denb", tag="denb")
                for hi in range(2):
                    nc.vector.tensor_scalar_mul(
                        den_b[:, hi * QB:(hi + 1) * QB], zps[hi][64:65, :], 1.0)
                pending_ep[0] = (zps, q0p, den_b)

            def emit_epilogue_a1():
                st = pending_ep[0]
                if st is None or len(st) != 3:
                    return
                zps, q0p, den_b = st
                rbs = []
                db_ps = sc_pool.tile([128, 2 * QB], f32, tag="sc")
                for hi in range(2):
                    nc.tensor.matmul(
                        db_ps[0:64, hi * QB:(hi + 1) * QB], lhsT=ones1[0:1, 0:64],
                        rhs=den_b[:, hi * QB:(hi + 1) * QB],
                        start=True, stop=True,
                    )
                    rb = ep_pool.tile([64, QB], f32, name=f"rb{hi}", tag=f"rb{hi}")
                    nc.vector.reciprocal_approx_fast(rb[:], db_ps[0:64, hi * QB:(hi + 1) * QB])
                    rbs.append(rb)
                pending_ep[0] = (zps, q0p, den_b, rbs)

            def emit_epilogue_b():
                st = pending_ep[0]
                if st is None or len(st) != 4:
                    return
                zps, q0p, _, rbs = st
                for hi in range(2):
                    nc.vector.tensor_tensor(
                        zt[hi * 64:(hi + 1) * 64, q0p: q0p + QB],
                        zps[hi][0:64, :], rbs[hi][:],
                        op=Alu.mult)
                pending_ep[0] = None

            def emit_scores(p_, qb_, kc_):
                # scores pair + exp + (exp-1)*m for (p_, qb_, kc_); returns e2
                ms_ = (qb_ % 2) * KC * QB
                q0_ = p_ * QW + qb_ * QB
                sc2 = sc_pool.tile([128, 2 * QB], f32, tag="sc")
                for hi in range(2):
                    r0, r1 = hi * 64, (hi + 1) * 64
                    nc.tensor.matmul(
                        sc2[:, hi * QB:(hi + 1) * QB],
                        lhsT=kt[r0:r1, p_ * SK + kc_ * 128: p_ * SK + (kc_ + 1) * 128],
                        rhs=qt_[r0:r1, q0_: q0_ + QB],
                        start=True, stop=True,
                    )
                e2 = att_pool.tile([128, 2 * QB], b16, tag="e")
                nc.scalar.activation(e2[:], sc2[:], Exp)
                m_blk = m_sb[:, ms_ + kc_ * QB: ms_ + (kc_ + 1) * QB]
                m_rep = m_blk.unsqueeze(1).broadcast_to([128, 2, QB])
                e3 = e2[:].rearrange("p (r c) -> p r c", r=2)
                nc.vector.scalar_tensor_tensor(
                    e3, e3, 1.0, m_rep, op0=Alu.subtract, op1=Alu.mult)
                return e2

            prefetched = [None]   # e2 of the next block's kc==0, if emitted
            NQB = QW // QB
            for qb in range(NQB):
                if qb + 1 < NQB:
                    dma_mask(qb + 1)
                for p in range(NPAIR):
                    q0 = p * QW + qb * QB
                    zt_ps = [
                        zt_pool.tile([128, QB], f32, name=f"ztp{hi}", tag=f"ztp{hi}")
                        for hi in range(2)
                    ]
                    for kc in range(KC):
                        if kc == 0 and prefetched[0] is not None:
                            e2 = prefetched[0]
                            prefetched[0] = None
                        else:
                            e2 = emit_scores(p, qb, kc)
                        if kc == 14:
                            # prefetch next block's first score tile so its
                            # e2m is ready before the boundary (no PE bubble)
                            if p + 1 < NPAIR:
                                prefetched[0] = emit_scores(p + 1, qb, 0)
                            elif qb + 1 < NQB:
                                prefetched[0] = emit_scores(0, qb + 1, 0)
                        if kc == 0:
                            # zT init: [Vcolsum_h | S] broadcast along q
                            for hi in range(2):
                                nc.tensor.matmul(
                                    zt_ps[hi][0:VW, :],
                                    lhsT=vch_sb[0:1, (2 * p + hi) * VW:(2 * p + hi + 1) * VW],
                                    rhs=onesr[:],
                                    start=True, stop=False,
                                )
                        for hi in range(2):
                            h = 2 * p + hi
                            nc.tensor.matmul(
                                zt_ps[hi][0:VW, :],
                                lhsT=vp[:, kc * HL * VW + h * VW: kc * HL * VW + (h + 1) * VW],
                                rhs=e2[:, hi * QB:(hi + 1) * QB],
                                start=False, stop=(kc == KC - 1),
                            )
                        if kc == 0:
                            emit_epilogue_a0()  # previous block: den rows out
                        elif kc == 1:
                            emit_epilogue_a1()  # previous block: bcast+recip
                        elif kc == 3:
                            emit_epilogue_b()   # previous block: normalize
                    pending_ep[0] = (zt_ps, q0)
            emit_epilogue_a0()
            emit_epilogue_a1()
            emit_epilogue_b()

        # ---------------- output projection (partial: our heads only) ------
        with (
            tc.tile_pool(name="wo_ps", space="PSUM", bufs=2) as wo_pool,
            tc.tile_pool(name="out_sb", bufs=2) as out_pool,
        ):
            for jq in range(QW // 128):
                o_sb = out_pool.tile([128, D], f32, tag="o")
                for n in range(2):
                    ps = wo_pool.tile([128, 512], f32, tag="wo")
                    for p in range(NPAIR):
                        nc.tensor.matmul(
                            ps[:],
                            lhsT=zt[:, p * QW + jq * 128: p * QW + (jq + 1) * 128],
                            rhs=wo_sb[:, p * D + n * 512: p * D + (n + 1) * 512],
                            start=(p == 0), stop=(p == NPAIR - 1),
                        )
                    nc.vector.scalar_tensor_tensor(
                        o_sb[:, n * 512:(n + 1) * 512], ps[:], 0.0,
                        bob_sb[:, n * 512:(n + 1) * 512],
                        op0=Alu.bypass, op1=Alu.add,
                    )
                nc.sync.dma_start(d_out[jq * 128:(jq + 1) * 128, :], o_sb[:])

        for f in reversed(_keep):
            f()

    nc.compile()
    return nc


def get_nc():
    global _CACHED_NC
    if _CACHED_NC is None:
        _CACHED_NC = _build_nc()
    return _CACHED_NC


def _prep_in_maps(x_v, x_k, x_q, mask, Wq, bq, Wk, bk, Wv, bv, Wo, bo):
    """Host-side shard + layout prep (numpy only)."""
    per_batch = []
    for b in range(B):
        xq_t = np.ascontiguousarray(x_q[b].T).astype(BF16)
        xk_t = np.ascontiguousarray(x_k[b].T).astype(BF16)
        xv_t = np.ascontiguousarray(x_v[b].T).astype(BF16)
        m_t = np.ascontiguousarray(mask[b].T.astype(np.float32)).astype(BF16)
        xvsum = x_v[b].sum(axis=0, dtype=np.float64)
        per_batch.append((xq_t, xk_t, xv_t, m_t, xvsum))

    per_half = []
    for hh in range(2):
        hs = hh * HL
        wq_f = (np.transpose(Wq[hs:hs + HL], (1, 0, 2)).reshape(D, 512) / 8.0).astype(BF16)
        wk_f = np.transpose(Wk[hs:hs + HL], (1, 0, 2)).reshape(D, 512).astype(BF16)
        wv_f = np.transpose(Wv[hs:hs + HL], (1, 0, 2)).reshape(D, 512).astype(BF16)
        wo_f = np.ascontiguousarray(
            Wo[hh * 512:(hh + 1) * 512].reshape(NPAIR, 128, D).transpose(1, 0, 2).reshape(128, NPAIR * D)
        ).astype(BF16)
        bq_f = np.ascontiguousarray((bq[hs:hs + HL].reshape(NPAIR, 128) / 8.0).T).astype(np.float32)
        bk_f = np.ascontiguousarray(bk[hs:hs + HL].reshape(NPAIR, 128).T).astype(np.float32)
        bv_f = bv[hs:hs + HL].reshape(1, 512).astype(BF16)
        if hh == 0:
            bob = np.ascontiguousarray(np.broadcast_to(bo.reshape(1, D), (128, D)), dtype=np.float32)
        else:
            bob = np.zeros((128, D), np.float32)
        per_half.append((wq_f, wk_f, wv_f, wo_f, bq_f, bk_f, bv_f, bob))

    in_maps = []
    for c in range(N_CORES):
        b, hh = c // 2, c % 2
        hs = hh * HL
        xq_t, xk_t, xv_t, m_t, xvsum = per_batch[b]
        wq_f, wk_f, wv_f, wo_f, bq_f, bk_f, bv_f, bob = per_half[hh]
        # vch[h] = [ (xv.sum(0) @ Wv[h] + S*bv[h])(64) | S ], exact on host
        vcs = (np.einsum("d,hde->he", xvsum, Wv[hs:hs + HL].astype(np.float64))
               + S * bv[hs:hs + HL].astype(np.float64))
        vch = np.zeros((1, HL * VW), np.float32)
        for h in range(HL):
            vch[0, h * VW: h * VW + 64] = vcs[h]
            vch[0, h * VW + 64] = float(S)
        in_maps.append({
            "xqt": xq_t, "xkt": xk_t, "xvt": xv_t, "m": m_t,
            "wq": wq_f, "wk": wk_f, "wv": wv_f, "wo": wo_f,
            "bq": bq_f, "bk": bk_f, "bv": bv_f,
            "vch": vch.astype(BF16), "bob": bob,
        })
    return in_maps


def _install_axon_ntff_hook():
    """Recreate antenv.axon_hooks and install the ctypes NTFF hook."""
    import types

    if "antenv.axon_hooks" in sys.modules:
        return
    import antenv

    mod = types.ModuleType("antenv.axon_hooks")
    _hook = [None]
    mod.set_axon_ntff_profile_hook = lambda h: _hook.__setitem__(0, h)
    mod.get_axon_ntff_profile_hook = lambda: _hook[0]
    sys.modules["antenv.axon_hooks"] = mod
    antenv.axon_hooks = mod
    try:
        sys.path.insert(0, "/root/.axon_site")
        from trn_agent_boot.trn_boot import _ntff_profile_via_ctypes

        mod.set_axon_ntff_profile_hook(
            _ntff_profile_via_ctypes("/opt/axon/libaxon_pjrt.so")
        )
    except Exception as e:  # degrade to no-trace
        print(f"ntff hook install failed: {e}", file=sys.stderr)


def run(trace=False, **inputs):
    if trace:
        _install_axon_ntff_hook()
    nc = get_nc()
    in_maps = _prep_in_maps(**inputs)
    res = run_bass_kernel_spmd(nc, in_maps, core_ids=list(range(N_CORES)), trace=trace)
    out = np.zeros((B, S, D), np.float32)
    for b in range(B):
        out[b] = res.results[2 * b]["out"] + res.results[2 * b + 1]["out"]
    return out, res


def kernel(**inputs):
    out, _ = run(trace=False, **inputs)
    return out


# revision 11
# speedup vs baseline: 1.1982x; 1.1982x over previous
"""Multi-head attention Trainium2 kernel v3 (8 NeuronCores, SPMD).

Problem: B=4, S=2048, D=1024, H=16, HD=64 dense MHA with 0/1 mask applied
to scores BEFORE softmax (masked score -> 0, so exp -> 1).

Sharding: core c handles batch b = c//2 and HEAD-half hh = c%2 (8 heads
= 4 pairs), ALL 2048 queries and keys -> no K/V projection duplication.
Each core emits a PARTIAL output (its heads' z @ Wo rows); the host adds
the two partials per batch (bo included on hh=0 only).

Mask/softmax factoring:  E = m*exp(s) + (1-m) = (exp(s)-1)*m + 1.
Per kc chunk the Scalar engine computes exp(s) (unmasked), ONE DVE
scalar_tensor_tensor forms (exp(s)-1)*m in bf16 at the 2x DVE rate (the
mask block is read twice through a 0-stride broadcast AP to cover both
heads), and the attn@V matmuls consume that. The "+1" term is exact and
mask-independent: sum_k 1*V'[k,e] = [Vcolsum_h | S], computed on the
HOST and injected as the PSUM *initialization* of each zT accumulator
via a K=1 matmul (lhsT = vch column, rhs = ones). The ones column of V'
then yields the full denominator in zT row 64 with no extra work.

P-state discipline: the PE drops from 2.4GHz to 1.2GHz after ~1us of
idle and needs ~3us of continuous work to ramp back. The attention loop
keeps every PE gap under ~0.5us: zT PSUM accumulators are double
buffered (8/8 banks used), and each block's epilogue (den copy ->
broadcast -> fast reciprocal -> normalize) is EMITTED two iterations
into the NEXT block so it fills DVE/PE slack instead of stalling the
queues at block boundaries.
"""

import sys

sys.path.insert(0, "/opt/trn_rl_repo")

import numpy as np
import ml_dtypes

import concourse.bass as bass
import concourse.mybir as mybir
import concourse.tile as tile
from concourse import bacc
from concourse.bass_utils import run_bass_kernel_spmd

BF16 = ml_dtypes.bfloat16

B, S, D, H, HD = 4, 2048, 1024, 16, 64
HL = 8             # local heads per core
NPAIR = 4          # local head pairs
SK = 2048          # keys
QW = 2048          # queries per core (all)
KC = 16            # key chunks of 128
DC = 8             # contraction chunks of 128 over D
QB = 512           # query block per head in the paired scores tile
VW = 65            # V width incl. ones column
N_CORES = 8

_CACHED_NC = None


def _build_nc():
    dt = mybir.dt
    f32, b16 = dt.float32, dt.bfloat16
    Copy = mybir.ActivationFunctionType.Copy
    Ident = mybir.ActivationFunctionType.Identity
    Exp = mybir.ActivationFunctionType.Exp
    Alu = mybir.AluOpType

    nc = bacc.Bacc("TRN2", target_bir_lowering=False, debug=False)

    d_xqt = nc.dram_tensor("xqt", [D, QW], b16, kind="ExternalInput").ap()
    d_xkt = nc.dram_tensor("xkt", [D, SK], b16, kind="ExternalInput").ap()
    d_xvt = nc.dram_tensor("xvt", [D, SK], b16, kind="ExternalInput").ap()
    d_m = nc.dram_tensor("m", [SK, QW], b16, kind="ExternalInput").ap()
    d_wq = nc.dram_tensor("wq", [D, 512], b16, kind="ExternalInput").ap()
    d_wk = nc.dram_tensor("wk", [D, 512], b16, kind="ExternalInput").ap()
    d_wv = nc.dram_tensor("wv", [D, 512], b16, kind="ExternalInput").ap()
    d_wo = nc.dram_tensor("wo", [128, NPAIR * D], b16, kind="ExternalInput").ap()
    d_bq = nc.dram_tensor("bq", [128, NPAIR], f32, kind="ExternalInput").ap()
    d_bk = nc.dram_tensor("bk", [128, NPAIR], f32, kind="ExternalInput").ap()
    d_bv = nc.dram_tensor("bv", [1, 512], b16, kind="ExternalInput").ap()
    # per pair/head: [Vcolsum_h(64) | S] columns for the zT PSUM init
    d_vch = nc.dram_tensor("vch", [1, NPAIR * 2 * VW], b16, kind="ExternalInput").ap()
    d_bob = nc.dram_tensor("bob", [128, D], f32, kind="ExternalInput").ap()
    d_out = nc.dram_tensor("out", [QW, D], f32, kind="ExternalOutput").ap()

    with tile.TileContext(nc) as tc:
        _keep = []

        def single(shape, dtype, name):
            t, free = tc.tile(shape, dtype, name=name)
            done = [False]

            def free_once():
                if not done[0]:
                    done[0] = True
                    free()

            _keep.append(free_once)
            return t, free_once

        # ---------------- persistent SBUF tiles ----------------
        kt, _ = single([128, NPAIR * SK], b16, "kt")       # [pair-e, k]
        qt_, _ = single([128, NPAIR * QW], b16, "qt")      # [pair-e, q]
        vp, _ = single([128, KC * HL * VW], b16, "vp")     # [k-chunk, h*65]
        # mask ring: 2 slots of [k-chunk, one qb column block]
        m_sb, _ = single([128, 2 * KC * QB], b16, "m_sb")
        wo_sb, _ = single([128, NPAIR * D], b16, "wo_sb")
        zt, _ = single([128, NPAIR * QW], b16, "zt")       # [pair-he, q]
        ones1, _ = single([1, 128], b16, "ones1")
        onesr, _ = single([1, 512], b16, "onesr")
        bqp, _ = single([128, NPAIR], f32, "bqp")
        bkp, _ = single([128, NPAIR], f32, "bkp")
        bvr, _ = single([1, 512], b16, "bvr")
        vch_sb, _ = single([1, NPAIR * 2 * VW], b16, "vch_sb")
        bob_sb, _ = single([128, D], f32, "bob_sb")

        nc.vector.memset(ones1[:], 1.0)
        nc.vector.memset(onesr[:], 1.0)
        nc.vector.memset(vp[:, 64::65], 1.0)

        nc.sync.dma_start(bqp[:], d_bq[:])
        nc.sync.dma_start(bkp[:], d_bk[:])
        nc.sync.dma_start(bvr[:], d_bv[:])
        nc.sync.dma_start(vch_sb[:], d_vch[:])
        nc.sync.dma_start(bob_sb[:], d_bob[:])
        nc.sync.dma_start(wo_sb[:], d_wo[:])

        # Phase-chained staging buffers (LIFO frees).
        xk_sb, xk_free = single([128, DC * SK], b16, "xk_sb")
        wk_sb, wk_free = single([128, DC * 512], b16, "wk_sb")
        xv_sb, xv_free = single([128, DC * SK], b16, "xv_sb")
        wv_sb, wv_free = single([128, DC * 512], b16, "wv_sb")

        for dc in range(DC):
            nc.sync.dma_start(xv_sb[:, dc * SK:(dc + 1) * SK], d_xvt[dc * 128:(dc + 1) * 128, :])
            nc.sync.dma_start(wv_sb[:, dc * 512:(dc + 1) * 512], d_wv[dc * 128:(dc + 1) * 128, :])
        # mask for the first query block into ring slot 0, then K staging
        def dma_mask(qb):
            s = (qb % 2) * KC * QB
            for kc in range(KC):
                nc.sync.dma_start(
                    m_sb[:, s + kc * QB: s + (kc + 1) * QB],
                    d_m[kc * 128:(kc + 1) * 128, qb * QB:(qb + 1) * QB],
                )

        dma_mask(0)
        for dc in range(DC):
            nc.sync.dma_start(xk_sb[:, dc * SK:(dc + 1) * SK], d_xkt[dc * 128:(dc + 1) * 128, :])
            nc.sync.dma_start(wk_sb[:, dc * 512:(dc + 1) * 512], d_wk[dc * 128:(dc + 1) * 128, :])

        # ---------------- projections ----------------
        with tc.tile_pool(name="proj_ps", space="PSUM", bufs=4) as proj_pool:
            # V proj: V'[k, h*65+e] (+ones col)
            for sc in range(KC):
                ps = proj_pool.tile([128, 512], f32, tag="ps")
                nc.tensor.matmul(  # bias: ones[k] x bv[he]
                    ps[:], lhsT=ones1[:, 0:128], rhs=bvr[:],
                    start=True, stop=False,
                )
                for dc in range(DC):
                    nc.tensor.matmul(
                        ps[:],
                        lhsT=xv_sb[:, dc * SK + sc * 128: dc * SK + (sc + 1) * 128],
                        rhs=wv_sb[:, dc * 512:(dc + 1) * 512],
                        start=False, stop=(dc == DC - 1),
                    )
                o3 = vp[:, sc * HL * VW: (sc + 1) * HL * VW]
                o3 = o3.rearrange("p (h c) -> p h c", h=HL)[:, :, 0:64]
                i3 = ps[:].rearrange("p (h c) -> p h c", h=HL)
                nc.scalar.activation(o3, i3, Copy)
            wv_free()
            xv_free()

            # stage Q inputs now so their DMA overlaps K projection
            xq_sb, xq_free = single([128, DC * QW], b16, "xq_sb")
            wq_sb, wq_free = single([128, DC * 512], b16, "wq_sb")
            for dc in range(DC):
                nc.sync.dma_start(xq_sb[:, dc * QW:(dc + 1) * QW], d_xqt[dc * 128:(dc + 1) * 128, :])
                nc.sync.dma_start(wq_sb[:, dc * 512:(dc + 1) * 512], d_wq[dc * 128:(dc + 1) * 128, :])

            # K proj -> KT [pair-e 128, k]
            for p in range(NPAIR):
                for ns in range(SK // 512):
                    ps = proj_pool.tile([128, 512], f32, tag="ps")
                    for dc in range(DC):
                        nc.tensor.matmul(
                            ps[:],
                            lhsT=wk_sb[:, dc * 512 + p * 128: dc * 512 + (p + 1) * 128],
                            rhs=xk_sb[:, dc * SK + ns * 512: dc * SK + (ns + 1) * 512],
                            start=(dc == 0), stop=(dc == DC - 1),
                        )
                    nc.scalar.activation(
                        kt[:, p * SK + ns * 512: p * SK + (ns + 1) * 512],
                        ps[:], Ident, bias=bkp[:, p: p + 1],
                    )
            # Q proj -> QT [pair-e 128, q]
            for p in range(NPAIR):
                for ns in range(QW // 512):
                    ps = proj_pool.tile([128, 512], f32, tag="ps")
                    for dc in range(DC):
                        nc.tensor.matmul(
                            ps[:],
                            lhsT=wq_sb[:, dc * 512 + p * 128: dc * 512 + (p + 1) * 128],
                            rhs=xq_sb[:, dc * QW + ns * 512: dc * QW + (ns + 1) * 512],
                            start=(dc == 0), stop=(dc == DC - 1),
                        )
                    nc.scalar.activation(
                        qt_[:, p * QW + ns * 512: p * QW + (ns + 1) * 512],
                        ps[:], Ident, bias=bqp[:, p: p + 1],
                    )
            wq_free()
            xq_free()
            wk_free()
            xk_free()

        # ---------------- attention ----------------
        with (
            tc.tile_pool(name="sc_ps", space="PSUM", bufs=2) as sc_pool,
            tc.tile_pool(name="zt_ps", space="PSUM", bufs=2) as zt_pool,
            tc.tile_pool(name="att_sb", bufs=4) as att_pool,
            tc.tile_pool(name="ep_sb", bufs=2) as ep_pool,
        ):
            pending_ep = [None]

            def emit_epilogue_a0():
                # den rows -> SBUF early (kc==0 slot) so the later broadcast
                # +reciprocal hold the borrowed scores-ring PSUM slot only
                # briefly (PE bubble stays under the p-state drop threshold).
                st = pending_ep[0]
                if st is None or len(st) != 2:
                    return
                zps, q0p = st
                den_b = ep_pool.tile([1, 2 * QB], b16, name="denb", tag="denb")
                for hi in range(2):
                    nc.vector.tensor_scalar_mul(
                        den_b[:, hi * QB:(hi + 1) * QB], zps[hi][64:65, :], 1.0)
                pending_ep[0] = (zps, q0p, den_b)

            def emit_epilogue_a1():
                st = pending_ep[0]
                if st is None or len(st) != 3:
                    return
                zps, q0p, den_b = st
                rbs = []
                db_ps = sc_pool.tile([128, 2 * QB], f32, tag="sc")
                for hi in range(2):
                    nc.tensor.matmul(
                        db_ps[0:64, hi * QB:(hi + 1) * QB], lhsT=ones1[0:1, 0:64],
                        rhs=den_b[:, hi * QB:(hi + 1) * QB],
                        start=True, stop=True,
                    )
                    rb = ep_pool.tile([64, QB], f32, name=f"rb{hi}", tag=f"rb{hi}")
                    nc.vector.reciprocal_approx_fast(rb[:], db_ps[0:64, hi * QB:(hi + 1) * QB])
                    rbs.append(rb)
                pending_ep[0] = (zps, q0p, den_b, rbs)

            def emit_epilogue_b():
                st = pending_ep[0]
                if st is None or len(st) != 4:
                    return
                zps, q0p, _, rbs = st
                for hi in range(2):
                    nc.vector.tensor_tensor(
                        zt[hi * 64:(hi + 1) * 64, q0p: q0p + QB],
                        zps[hi][0:64, :], rbs[hi][:],
                        op=Alu.mult)
                pending_ep[0] = None

            for qb in range(QW // QB):
                if qb + 1 < QW // QB:
                    dma_mask(qb + 1)
                ms = (qb % 2) * KC * QB
                for p in range(NPAIR):
                    q0 = p * QW + qb * QB
                    zt_ps = [
                        zt_pool.tile([128, QB], f32, name=f"ztp{hi}", tag=f"ztp{hi}")
                        for hi in range(2)
                    ]
                    for kc in range(KC):
                        sc2 = sc_pool.tile([128, 2 * QB], f32, tag="sc")
                        for hi in range(2):
                            r0, r1 = hi * 64, (hi + 1) * 64
                            nc.tensor.matmul(
                                sc2[:, hi * QB:(hi + 1) * QB],
                                lhsT=kt[r0:r1, p * SK + kc * 128: p * SK + (kc + 1) * 128],
                                rhs=qt_[r0:r1, q0: q0 + QB],
                                start=True, stop=True,
                            )
                        e2 = att_pool.tile([128, 2 * QB], b16, tag="e")
                        nc.scalar.activation(e2[:], sc2[:], Exp)
                        # (exp(s)-1)*m; both halves read the same m block
                        m_blk = m_sb[:, ms + kc * QB: ms + (kc + 1) * QB]
                        m_rep = m_blk.unsqueeze(1).broadcast_to([128, 2, QB])
                        e3 = e2[:].rearrange("p (r c) -> p r c", r=2)
                        nc.vector.scalar_tensor_tensor(
                            e3, e3, 1.0, m_rep, op0=Alu.subtract, op1=Alu.mult)
                        if kc == 0:
                            # zT init: [Vcolsum_h | S] broadcast along q
                            for hi in range(2):
                                nc.tensor.matmul(
                                    zt_ps[hi][0:VW, :],
                                    lhsT=vch_sb[0:1, (2 * p + hi) * VW:(2 * p + hi + 1) * VW],
                                    rhs=onesr[:],
                                    start=True, stop=False,
                                )
                        for hi in range(2):
                            h = 2 * p + hi
                            nc.tensor.matmul(
                                zt_ps[hi][0:VW, :],
                                lhsT=vp[:, kc * HL * VW + h * VW: kc * HL * VW + (h + 1) * VW],
                                rhs=e2[:, hi * QB:(hi + 1) * QB],
                                start=False, stop=(kc == KC - 1),
                            )
                        if kc == 0:
                            emit_epilogue_a0()  # previous block: den rows out
                        elif kc == 1:
                            emit_epilogue_a1()  # previous block: bcast+recip
                        elif kc == 3:
                            emit_epilogue_b()   # previous block: normalize
                    pending_ep[0] = (zt_ps, q0)
            emit_epilogue_a0()
            emit_epilogue_a1()
            emit_epilogue_b()

        # ---------------- output projection (partial: our heads only) ------
        with (
            tc.tile_pool(name="wo_ps", space="PSUM", bufs=2) as wo_pool,
            tc.tile_pool(name="out_sb", bufs=2) as out_pool,
        ):
            for jq in range(QW // 128):
                o_sb = out_pool.tile([128, D], f32, tag="o")
                for n in range(2):
                    ps = wo_pool.tile([128, 512], f32, tag="wo")
                    for p in range(NPAIR):
                        nc.tensor.matmul(
                            ps[:],
                            lhsT=zt[:, p * QW + jq * 128: p * QW + (jq + 1) * 128],
                            rhs=wo_sb[:, p * D + n * 512: p * D + (n + 1) * 512],
                            start=(p == 0), stop=(p == NPAIR - 1),
                        )
                    nc.vector.scalar_tensor_tensor(
                        o_sb[:, n * 512:(n + 1) * 512], ps[:], 0.0,
                        bob_sb[:, n * 512:(n + 1) * 512],
                        op0=Alu.bypass, op1=Alu.add,
                    )
                nc.sync.dma_start(d_out[jq * 128:(jq + 1) * 128, :], o_sb[:])

        for f in reversed(_keep):
            f()

    nc.compile()
    return nc


def get_nc():
    global _CACHED_NC
    if _CACHED_NC is None:
        _CACHED_NC = _build_nc()
    return _CACHED_NC


def _prep_in_maps(x_v, x_k, x_q, mask, Wq, bq, Wk, bk, Wv, bv, Wo, bo):
    """Host-side shard + layout prep (numpy only)."""
    per_batch = []
    for b in range(B):
        xq_t = np.ascontiguousarray(x_q[b].T).astype(BF16)
        xk_t = np.ascontiguousarray(x_k[b].T).astype(BF16)
        xv_t = np.ascontiguousarray(x_v[b].T).astype(BF16)
        m_t = np.ascontiguousarray(mask[b].T.astype(np.float32)).astype(BF16)
        xvsum = x_v[b].sum(axis=0, dtype=np.float64)
        per_batch.append((xq_t, xk_t, xv_t, m_t, xvsum))

    per_half = []
    for hh in range(2):
        hs = hh * HL
        wq_f = (np.transpose(Wq[hs:hs + HL], (1, 0, 2)).reshape(D, 512) / 8.0).astype(BF16)
        wk_f = np.transpose(Wk[hs:hs + HL], (1, 0, 2)).reshape(D, 512).astype(BF16)
        wv_f = np.transpose(Wv[hs:hs + HL], (1, 0, 2)).reshape(D, 512).astype(BF16)
        wo_f = np.ascontiguousarray(
            Wo[hh * 512:(hh + 1) * 512].reshape(NPAIR, 128, D).transpose(1, 0, 2).reshape(128, NPAIR * D)
        ).astype(BF16)
        bq_f = np.ascontiguousarray((bq[hs:hs + HL].reshape(NPAIR, 128) / 8.0).T).astype(np.float32)
        bk_f = np.ascontiguousarray(bk[hs:hs + HL].reshape(NPAIR, 128).T).astype(np.float32)
        bv_f = bv[hs:hs + HL].reshape(1, 512).astype(BF16)
        if hh == 0:
            bob = np.ascontiguousarray(np.broadcast_to(bo.reshape(1, D), (128, D)), dtype=np.float32)
        else:
            bob = np.zeros((128, D), np.float32)
        per_half.append((wq_f, wk_f, wv_f, wo_f, bq_f, bk_f, bv_f, bob))

    in_maps = []
    for c in range(N_CORES):
        b, hh = c // 2, c % 2
        hs = hh * HL
        xq_t, xk_t, xv_t, m_t, xvsum = per_batch[b]
        wq_f, wk_f, wv_f, wo_f, bq_f, bk_f, bv_f, bob = per_half[hh]
        # vch[h] = [ (xv.sum(0) @ Wv[h] + S*bv[h])(64) | S ], exact on host
        vcs = (np.einsum("d,hde->he", xvsum, Wv[hs:hs + HL].astype(np.float64))
               + S * bv[hs:hs + HL].astype(np.float64))
        vch = np.zeros((1, HL * VW), np.float32)
        for h in range(HL):
            vch[0, h * VW: h * VW + 64] = vcs[h]
            vch[0, h * VW + 64] = float(S)
        in_maps.append({
            "xqt": xq_t, "xkt": xk_t, "xvt": xv_t, "m": m_t,
            "wq": wq_f, "wk": wk_f, "wv": wv_f, "wo": wo_f,
            "bq": bq_f, "bk": bk_f, "bv": bv_f,
            "vch": vch.astype(BF16), "bob": bob,
        })
    return in_maps


def _install_axon_ntff_hook():
    """Recreate antenv.axon_hooks and install the ctypes NTFF hook."""
    import types

    if "antenv.axon_hooks" in sys.modules:
        return
    import antenv

    mod = types.ModuleType("antenv.axon_hooks")
    _hook = [None]
    mod.set_axon_ntff_profile_hook = lambda h: _hook.__setitem__(0, h)
    mod.get_axon_ntff_profile_hook = lambda: _hook[0]
    sys.modules["antenv.axon_hooks"] = mod
    antenv.axon_hooks = mod
    try:
        sys.path.insert(0, "/root/.axon_site")
        from trn_agent_boot.trn_boot import _ntff_profile_via_ctypes

        mod.set_axon_ntff_profile_hook(
            _ntff_profile_via_ctypes("/opt/axon/libaxon_pjrt.so")
        )
    except Exception as e:  # degrade to no-trace
        print(f"ntff hook install failed: {e}", file=sys.stderr)


def run(trace=False, **inputs):
    if trace:
        _install_axon_ntff_hook()
    nc = get_nc()
    in_maps = _prep_in_maps(**inputs)
    res = run_bass_kernel_spmd(nc, in_maps, core_ids=list(range(N_CORES)), trace=trace)
    out = np.zeros((B, S, D), np.float32)
    for b in range(B):
        out[b] = res.results[2 * b]["out"] + res.results[2 * b + 1]["out"]
    return out, res


def kernel(**inputs):
    out, _ = run(trace=False, **inputs)
    return out


# revision 12
# speedup vs baseline: 1.2439x; 1.0381x over previous
"""Multi-head attention Trainium2 kernel v3 (8 NeuronCores, SPMD).

Problem: B=4, S=2048, D=1024, H=16, HD=64 dense MHA with 0/1 mask applied
to scores BEFORE softmax (masked score -> 0, so exp -> 1).

Sharding: core c handles batch b = c//2 and HEAD-half hh = c%2 (8 heads
= 4 pairs), ALL 2048 queries and keys -> no K/V projection duplication.
Each core emits a PARTIAL output (its heads' z @ Wo rows); the host adds
the two partials per batch (bo included on hh=0 only).

Mask/softmax factoring:  E = m*exp(s) + (1-m) = (exp(s)-1)*m + 1.
Per kc chunk the Scalar engine computes exp(s) (unmasked), ONE DVE
scalar_tensor_tensor forms (exp(s)-1)*m in bf16 at the 2x DVE rate (the
mask block is read twice through a 0-stride broadcast AP to cover both
heads), and the attn@V matmuls consume that. The "+1" term is exact and
mask-independent: sum_k 1*V'[k,e] = [Vcolsum_h | S], computed on the
HOST and injected as the PSUM *initialization* of each zT accumulator
via a K=1 matmul (lhsT = vch column, rhs = ones). The ones column of V'
then yields the full denominator in zT row 64 with no extra work.

P-state discipline: the PE drops from 2.4GHz to 1.2GHz after ~1us of
idle and needs ~3us of continuous work to ramp back. The attention loop
keeps every PE gap under ~0.5us: zT PSUM accumulators are double
buffered (8/8 banks used), and each block's epilogue (den copy ->
broadcast -> fast reciprocal -> normalize) is EMITTED two iterations
into the NEXT block so it fills DVE/PE slack instead of stalling the
queues at block boundaries.
"""

import sys

sys.path.insert(0, "/opt/trn_rl_repo")

import numpy as np
import ml_dtypes

import concourse.bass as bass
import concourse.mybir as mybir
import concourse.tile as tile
from concourse import bacc
from concourse.bass_utils import run_bass_kernel_spmd

BF16 = ml_dtypes.bfloat16

B, S, D, H, HD = 4, 2048, 1024, 16, 64
HL = 8             # local heads per core
NPAIR = 4          # local head pairs
SK = 2048          # keys
QW = 2048          # queries per core (all)
KC = 16            # key chunks of 128
DC = 8             # contraction chunks of 128 over D
QB = 512           # query block per head in the paired scores tile
VW = 65            # V width incl. ones column
N_CORES = 8

_CACHED_NC = None


def _build_nc():
    dt = mybir.dt
    f32, b16 = dt.float32, dt.bfloat16
    Copy = mybir.ActivationFunctionType.Copy
    Ident = mybir.ActivationFunctionType.Identity
    Exp = mybir.ActivationFunctionType.Exp
    Alu = mybir.AluOpType

    nc = bacc.Bacc("TRN2", target_bir_lowering=False, debug=False)

    d_xqt = nc.dram_tensor("xqt", [D, QW], b16, kind="ExternalInput").ap()
    d_xkt = nc.dram_tensor("xkt", [D, SK], b16, kind="ExternalInput").ap()
    d_xvt = nc.dram_tensor("xvt", [D, SK], b16, kind="ExternalInput").ap()
    d_m = nc.dram_tensor("m", [SK, QW], b16, kind="ExternalInput").ap()
    d_wq = nc.dram_tensor("wq", [D, 512], b16, kind="ExternalInput").ap()
    d_wk = nc.dram_tensor("wk", [D, 512], b16, kind="ExternalInput").ap()
    d_wv = nc.dram_tensor("wv", [D, 512], b16, kind="ExternalInput").ap()
    d_wo = nc.dram_tensor("wo", [128, NPAIR * D], b16, kind="ExternalInput").ap()
    d_bq = nc.dram_tensor("bq", [128, NPAIR], f32, kind="ExternalInput").ap()
    d_bk = nc.dram_tensor("bk", [128, NPAIR], f32, kind="ExternalInput").ap()
    d_bv = nc.dram_tensor("bv", [1, 512], b16, kind="ExternalInput").ap()
    # per pair/head: [Vcolsum_h(64) | S] columns for the zT PSUM init
    d_vch = nc.dram_tensor("vch", [1, NPAIR * 2 * VW], b16, kind="ExternalInput").ap()
    d_bob = nc.dram_tensor("bob", [128, D], f32, kind="ExternalInput").ap()
    d_out = nc.dram_tensor("out", [QW, D], f32, kind="ExternalOutput").ap()

    with tile.TileContext(nc) as tc:
        _keep = []

        def single(shape, dtype, name):
            t, free = tc.tile(shape, dtype, name=name)
            done = [False]

            def free_once():
                if not done[0]:
                    done[0] = True
                    free()

            _keep.append(free_once)
            return t, free_once

        # ---------------- persistent SBUF tiles ----------------
        kt, _ = single([128, NPAIR * SK], b16, "kt")       # [pair-e, k]
        qt_, _ = single([128, NPAIR * QW], b16, "qt")      # [pair-e, q]
        vp, _ = single([128, KC * HL * VW], b16, "vp")     # [k-chunk, h*65]
        # mask ring: 2 slots of [k-chunk, one qb column block]
        m_sb, _ = single([128, 2 * KC * QB], b16, "m_sb")
        wo_sb, _ = single([128, NPAIR * D], b16, "wo_sb")
        zt, _ = single([128, NPAIR * QW], b16, "zt")       # [pair-he, q]
        ones1, _ = single([1, 128], b16, "ones1")
        onesr, _ = single([1, 512], b16, "onesr")
        bqp, _ = single([128, NPAIR], f32, "bqp")
        bkp, _ = single([128, NPAIR], f32, "bkp")
        bvr, _ = single([1, 512], b16, "bvr")
        vch_sb, _ = single([1, NPAIR * 2 * VW], b16, "vch_sb")
        bob_sb, _ = single([128, D], f32, "bob_sb")

        nc.vector.memset(ones1[:], 1.0)
        nc.vector.memset(onesr[:], 1.0)
        nc.vector.memset(vp[:, 64::65], 1.0)

        nc.sync.dma_start(bqp[:], d_bq[:])
        nc.sync.dma_start(bkp[:], d_bk[:])
        nc.sync.dma_start(bvr[:], d_bv[:])
        nc.sync.dma_start(vch_sb[:], d_vch[:])
        nc.sync.dma_start(bob_sb[:], d_bob[:])
        nc.sync.dma_start(wo_sb[:], d_wo[:])

        # Phase-chained staging buffers (LIFO frees).
        xk_sb, xk_free = single([128, DC * SK], b16, "xk_sb")
        wk_sb, wk_free = single([128, DC * 512], b16, "wk_sb")
        xv_sb, xv_free = single([128, DC * SK], b16, "xv_sb")
        wv_sb, wv_free = single([128, DC * 512], b16, "wv_sb")

        for dc in range(DC):
            nc.sync.dma_start(xv_sb[:, dc * SK:(dc + 1) * SK], d_xvt[dc * 128:(dc + 1) * 128, :])
            nc.sync.dma_start(wv_sb[:, dc * 512:(dc + 1) * 512], d_wv[dc * 128:(dc + 1) * 128, :])
        # mask for the first query block into ring slot 0, then K staging
        def dma_mask(qb):
            s = (qb % 2) * KC * QB
            for kc in range(KC):
                nc.sync.dma_start(
                    m_sb[:, s + kc * QB: s + (kc + 1) * QB],
                    d_m[kc * 128:(kc + 1) * 128, qb * QB:(qb + 1) * QB],
                )

        dma_mask(0)
        for dc in range(DC):
            nc.sync.dma_start(xk_sb[:, dc * SK:(dc + 1) * SK], d_xkt[dc * 128:(dc + 1) * 128, :])
            nc.sync.dma_start(wk_sb[:, dc * 512:(dc + 1) * 512], d_wk[dc * 128:(dc + 1) * 128, :])

        # ---------------- projections ----------------
        with tc.tile_pool(name="proj_ps", space="PSUM", bufs=6) as proj_pool:
            # V proj: V'[k, h*65+e] (+ones col)
            for sc in range(KC):
                ps = proj_pool.tile([128, 512], f32, tag="ps")
                nc.tensor.matmul(  # bias: ones[k] x bv[he]
                    ps[:], lhsT=ones1[:, 0:128], rhs=bvr[:],
                    start=True, stop=False,
                )
                for dc in range(DC):
                    nc.tensor.matmul(
                        ps[:],
                        lhsT=xv_sb[:, dc * SK + sc * 128: dc * SK + (sc + 1) * 128],
                        rhs=wv_sb[:, dc * 512:(dc + 1) * 512],
                        start=False, stop=(dc == DC - 1),
                    )
                o3 = vp[:, sc * HL * VW: (sc + 1) * HL * VW]
                o3 = o3.rearrange("p (h c) -> p h c", h=HL)[:, :, 0:64]
                i3 = ps[:].rearrange("p (h c) -> p h c", h=HL)
                nc.scalar.activation(o3, i3, Copy)
            wv_free()
            xv_free()

            # stage Q inputs now so their DMA overlaps K projection
            xq_sb, xq_free = single([128, DC * QW], b16, "xq_sb")
            wq_sb, wq_free = single([128, DC * 512], b16, "wq_sb")
            for dc in range(DC):
                nc.sync.dma_start(xq_sb[:, dc * QW:(dc + 1) * QW], d_xqt[dc * 128:(dc + 1) * 128, :])
                nc.sync.dma_start(wq_sb[:, dc * 512:(dc + 1) * 512], d_wq[dc * 128:(dc + 1) * 128, :])

            # K proj -> KT [pair-e 128, k]
            for p in range(NPAIR):
                for ns in range(SK // 512):
                    ps = proj_pool.tile([128, 512], f32, tag="ps")
                    for dc in range(DC):
                        nc.tensor.matmul(
                            ps[:],
                            lhsT=wk_sb[:, dc * 512 + p * 128: dc * 512 + (p + 1) * 128],
                            rhs=xk_sb[:, dc * SK + ns * 512: dc * SK + (ns + 1) * 512],
                            start=(dc == 0), stop=(dc == DC - 1),
                        )
                    nc.scalar.activation(
                        kt[:, p * SK + ns * 512: p * SK + (ns + 1) * 512],
                        ps[:], Ident, bias=bkp[:, p: p + 1],
                    )
            # Q proj -> QT [pair-e 128, q]
            for p in range(NPAIR):
                for ns in range(QW // 512):
                    ps = proj_pool.tile([128, 512], f32, tag="ps")
                    for dc in range(DC):
                        nc.tensor.matmul(
                            ps[:],
                            lhsT=wq_sb[:, dc * 512 + p * 128: dc * 512 + (p + 1) * 128],
                            rhs=xq_sb[:, dc * QW + ns * 512: dc * QW + (ns + 1) * 512],
                            start=(dc == 0), stop=(dc == DC - 1),
                        )
                    nc.scalar.activation(
                        qt_[:, p * QW + ns * 512: p * QW + (ns + 1) * 512],
                        ps[:], Ident, bias=bqp[:, p: p + 1],
                    )
            wq_free()
            xq_free()
            wk_free()
            xk_free()

        # ---------------- attention ----------------
        with (
            tc.tile_pool(name="sc_ps", space="PSUM", bufs=2) as sc_pool,
            tc.tile_pool(name="zt_ps", space="PSUM", bufs=2) as zt_pool,
            tc.tile_pool(name="att_sb", bufs=4) as att_pool,
            tc.tile_pool(name="ep_sb", bufs=2) as ep_pool,
        ):
            pending_ep = [None]

            def emit_epilogue_a0():
                # den rows -> SBUF early (kc==0 slot) so the later broadcast
                # +reciprocal hold the borrowed scores-ring PSUM slot only
                # briefly (PE bubble stays under the p-state drop threshold).
                st = pending_ep[0]
                if st is None or len(st) != 2:
                    return
                zps, q0p = st
                den_b = ep_pool.tile([1, 2 * QB], b16, name="denb", tag="denb")
                for hi in range(2):
                    nc.vector.tensor_scalar_mul(
                        den_b[:, hi * QB:(hi + 1) * QB], zps[hi][64:65, :], 1.0)
                pending_ep[0] = (zps, q0p, den_b)

            def emit_epilogue_a1():
                st = pending_ep[0]
                if st is None or len(st) != 3:
                    return
                zps, q0p, den_b = st
                rbs = []
                db_ps = sc_pool.tile([128, 2 * QB], f32, tag="sc")
                for hi in range(2):
                    nc.tensor.matmul(
                        db_ps[0:64, hi * QB:(hi + 1) * QB], lhsT=ones1[0:1, 0:64],
                        rhs=den_b[:, hi * QB:(hi + 1) * QB],
                        start=True, stop=True,
                    )
                    rb = ep_pool.tile([64, QB], f32, name=f"rb{hi}", tag=f"rb{hi}")
                    nc.vector.reciprocal_approx_fast(rb[:], db_ps[0:64, hi * QB:(hi + 1) * QB])
                    rbs.append(rb)
                pending_ep[0] = (zps, q0p, den_b, rbs)

            def emit_epilogue_b():
                st = pending_ep[0]
                if st is None or len(st) != 4:
                    return
                zps, q0p, _, rbs = st
                for hi in range(2):
                    nc.vector.tensor_tensor(
                        zt[hi * 64:(hi + 1) * 64, q0p: q0p + QB],
                        zps[hi][0:64, :], rbs[hi][:],
                        op=Alu.mult)
                pending_ep[0] = None

            for qb in range(QW // QB):
                if qb + 1 < QW // QB:
                    dma_mask(qb + 1)
                ms = (qb % 2) * KC * QB
                for p in range(NPAIR):
                    q0 = p * QW + qb * QB
                    zt_ps = [
                        zt_pool.tile([128, QB], f32, name=f"ztp{hi}", tag=f"ztp{hi}")
                        for hi in range(2)
                    ]
                    for kc in range(KC):
                        sc2 = sc_pool.tile([128, 2 * QB], f32, tag="sc")
                        for hi in range(2):
                            r0, r1 = hi * 64, (hi + 1) * 64
                            nc.tensor.matmul(
                                sc2[:, hi * QB:(hi + 1) * QB],
                                lhsT=kt[r0:r1, p * SK + kc * 128: p * SK + (kc + 1) * 128],
                                rhs=qt_[r0:r1, q0: q0 + QB],
                                start=True, stop=True,
                            )
                        e2 = att_pool.tile([128, 2 * QB], b16, tag="e")
                        nc.scalar.activation(e2[:], sc2[:], Exp)
                        # (exp(s)-1)*m; both halves read the same m block
                        m_blk = m_sb[:, ms + kc * QB: ms + (kc + 1) * QB]
                        m_rep = m_blk.unsqueeze(1).broadcast_to([128, 2, QB])
                        e3 = e2[:].rearrange("p (r c) -> p r c", r=2)
                        nc.vector.scalar_tensor_tensor(
                            e3, e3, 1.0, m_rep, op0=Alu.subtract, op1=Alu.mult)
                        if kc == 0:
                            # zT init: [Vcolsum_h | S] broadcast along q
                            for hi in range(2):
                                nc.tensor.matmul(
                                    zt_ps[hi][0:VW, :],
                                    lhsT=vch_sb[0:1, (2 * p + hi) * VW:(2 * p + hi + 1) * VW],
                                    rhs=onesr[:],
                                    start=True, stop=False,
                                )
                        for hi in range(2):
                            h = 2 * p + hi
                            nc.tensor.matmul(
                                zt_ps[hi][0:VW, :],
                                lhsT=vp[:, kc * HL * VW + h * VW: kc * HL * VW + (h + 1) * VW],
                                rhs=e2[:, hi * QB:(hi + 1) * QB],
                                start=False, stop=(kc == KC - 1),
                            )
                        if kc == 0:
                            emit_epilogue_a0()  # previous block: den rows out
                        elif kc == 1:
                            emit_epilogue_a1()  # previous block: bcast+recip
                        elif kc == 3:
                            emit_epilogue_b()   # previous block: normalize
                    pending_ep[0] = (zt_ps, q0)
            emit_epilogue_a0()
            emit_epilogue_a1()
            emit_epilogue_b()

        # ---------------- output projection (partial: our heads only) ------
        with (
            tc.tile_pool(name="wo_ps", space="PSUM", bufs=4) as wo_pool,
            tc.tile_pool(name="out_sb", bufs=3) as out_pool,
        ):
            for jq in range(QW // 128):
                o_sb = out_pool.tile([128, D], f32, tag="o")
                for n in range(2):
                    ps = wo_pool.tile([128, 512], f32, tag="wo")
                    for p in range(NPAIR):
                        nc.tensor.matmul(
                            ps[:],
                            lhsT=zt[:, p * QW + jq * 128: p * QW + (jq + 1) * 128],
                            rhs=wo_sb[:, p * D + n * 512: p * D + (n + 1) * 512],
                            start=(p == 0), stop=(p == NPAIR - 1),
                        )
                    nc.vector.scalar_tensor_tensor(
                        o_sb[:, n * 512:(n + 1) * 512], ps[:], 0.0,
                        bob_sb[:, n * 512:(n + 1) * 512],
                        op0=Alu.bypass, op1=Alu.add,
                    )
                nc.sync.dma_start(d_out[jq * 128:(jq + 1) * 128, :], o_sb[:])

        for f in reversed(_keep):
            f()

    nc.compile()
    return nc


def get_nc():
    global _CACHED_NC
    if _CACHED_NC is None:
        _CACHED_NC = _build_nc()
    return _CACHED_NC


def _prep_in_maps(x_v, x_k, x_q, mask, Wq, bq, Wk, bk, Wv, bv, Wo, bo):
    """Host-side shard + layout prep (numpy only)."""
    per_batch = []
    for b in range(B):
        xq_t = np.ascontiguousarray(x_q[b].T).astype(BF16)
        xk_t = np.ascontiguousarray(x_k[b].T).astype(BF16)
        xv_t = np.ascontiguousarray(x_v[b].T).astype(BF16)
        m_t = np.ascontiguousarray(mask[b].T.astype(np.float32)).astype(BF16)
        xvsum = x_v[b].sum(axis=0, dtype=np.float64)
        per_batch.append((xq_t, xk_t, xv_t, m_t, xvsum))

    per_half = []
    for hh in range(2):
        hs = hh * HL
        wq_f = (np.transpose(Wq[hs:hs + HL], (1, 0, 2)).reshape(D, 512) / 8.0).astype(BF16)
        wk_f = np.transpose(Wk[hs:hs + HL], (1, 0, 2)).reshape(D, 512).astype(BF16)
        wv_f = np.transpose(Wv[hs:hs + HL], (1, 0, 2)).reshape(D, 512).astype(BF16)
        wo_f = np.ascontiguousarray(
            Wo[hh * 512:(hh + 1) * 512].reshape(NPAIR, 128, D).transpose(1, 0, 2).reshape(128, NPAIR * D)
        ).astype(BF16)
        bq_f = np.ascontiguousarray((bq[hs:hs + HL].reshape(NPAIR, 128) / 8.0).T).astype(np.float32)
        bk_f = np.ascontiguousarray(bk[hs:hs + HL].reshape(NPAIR, 128).T).astype(np.float32)
        bv_f = bv[hs:hs + HL].reshape(1, 512).astype(BF16)
        if hh == 0:
            bob = np.ascontiguousarray(np.broadcast_to(bo.reshape(1, D), (128, D)), dtype=np.float32)
        else:
            bob = np.zeros((128, D), np.float32)
        per_half.append((wq_f, wk_f, wv_f, wo_f, bq_f, bk_f, bv_f, bob))

    in_maps = []
    for c in range(N_CORES):
        b, hh = c // 2, c % 2
        hs = hh * HL
        xq_t, xk_t, xv_t, m_t, xvsum = per_batch[b]
        wq_f, wk_f, wv_f, wo_f, bq_f, bk_f, bv_f, bob = per_half[hh]
        # vch[h] = [ (xv.sum(0) @ Wv[h] + S*bv[h])(64) | S ], exact on host
        vcs = (np.einsum("d,hde->he", xvsum, Wv[hs:hs + HL].astype(np.float64))
               + S * bv[hs:hs + HL].astype(np.float64))
        vch = np.zeros((1, HL * VW), np.float32)
        for h in range(HL):
            vch[0, h * VW: h * VW + 64] = vcs[h]
            vch[0, h * VW + 64] = float(S)
        in_maps.append({
            "xqt": xq_t, "xkt": xk_t, "xvt": xv_t, "m": m_t,
            "wq": wq_f, "wk": wk_f, "wv": wv_f, "wo": wo_f,
            "bq": bq_f, "bk": bk_f, "bv": bv_f,
            "vch": vch.astype(BF16), "bob": bob,
        })
    return in_maps


def _install_axon_ntff_hook():
    """Recreate antenv.axon_hooks and install the ctypes NTFF hook."""
    import types

    if "antenv.axon_hooks" in sys.modules:
        return
    import antenv

    mod = types.ModuleType("antenv.axon_hooks")
    _hook = [None]
    mod.set_axon_ntff_profile_hook = lambda h: _hook.__setitem__(0, h)
    mod.get_axon_ntff_profile_hook = lambda: _hook[0]
    sys.modules["antenv.axon_hooks"] = mod
    antenv.axon_hooks = mod
    try:
        sys.path.insert(0, "/root/.axon_site")
        from trn_agent_boot.trn_boot import _ntff_profile_via_ctypes

        mod.set_axon_ntff_profile_hook(
            _ntff_profile_via_ctypes("/opt/axon/libaxon_pjrt.so")
        )
    except Exception as e:  # degrade to no-trace
        print(f"ntff hook install failed: {e}", file=sys.stderr)


def run(trace=False, **inputs):
    if trace:
        _install_axon_ntff_hook()
    nc = get_nc()
    in_maps = _prep_in_maps(**inputs)
    res = run_bass_kernel_spmd(nc, in_maps, core_ids=list(range(N_CORES)), trace=trace)
    out = np.zeros((B, S, D), np.float32)
    for b in range(B):
        out[b] = res.results[2 * b]["out"] + res.results[2 * b + 1]["out"]
    return out, res


def kernel(**inputs):
    out, _ = run(trace=False, **inputs)
    return out
